# revision 37
# baseline (speedup 1.0000x reference)
"""GAT attention kernel for Trainium2 (Bass/Tile), 8-core data parallel.

Per-core math (2 examples each, N=256 items, D=64):
  e   = LayerNorm(emb);  ua = e[0] * e[2:]
  qk  = LeakyReLU(s_q_i + s_k_j + c);  alpha = softmax_j
  attention over value_ij = LN(ua_i * ua_j) collapsed via gram matrices:
    mu = UA@UA^T/D,  E2 = UA^2@UA^2^T/D,  invs = rsqrt(E2 - mu^2 + eps)
    att_i = g*(ua_i*Sn_i - cn_i) + b,  Sn = (beta~@UA)*rden,  cn = rowsum(ua*Sn)/D
  with beta~ = exp(qk^T)*invs (unnormalized), rden = 1/sum_j exp.
  out = LeakyReLU(concat([e0*e1], att))

Perf structure (v2):
  - all N^2 surfaces in bf16: PE matmuls 1-pass (vs fp32r 2-pass), DVE
    packed 2x/4x, half SBUF traffic. Scores/exp-inputs stay fp32.
  - single stacked [128,256] bf16 tile holds UA^T/sqrt(D) (rows 0:64) and
    (UA^2)^T/sqrt(D) (rows 64:128): mu- and E2-gram matmuls use disjoint
    PE row-groups (K=64) and run concurrently.
  - scores on PE: s_q broadcast via vq-replicated weights (one matmul),
    s_k + (s_i + c0) accumulated into one PSUM column per j-block using a
    stacked [iid_bc; ones] lhsT — no per-example row DMAs, no ACT row ops.
  - rsqrt as exp(-0.5*ln(var+eps)): Ln and Exp share one ACT table set, so
    exactly ONE ACT_TABLE_LOAD and no cross-example ordering barrier.
  - embedding-LN rsqrt on DVE (quake bit trick + 1 Newton step).
"""

import numpy as np

import concourse.bass as bass
from concourse import bacc
import concourse.mybir as mybir
import concourse.tile as tile
from concourse import masks
from concourse.bass_utils import run_bass_kernel_spmd

F32 = mybir.dt.float32
F32R = mybir.dt.float32r
BF16 = mybir.dt.bfloat16
I32 = mybir.dt.int32
ALU = mybir.AluOpType
ACTF = mybir.ActivationFunctionType
AX = mybir.AxisListType

B, NODE, D = 16, 258, 64
N = NODE - 2
N_CORES = 8
B_LOC = B // N_CORES
EPS = 1e-5
SLOPE = 0.01
OUT_ROWS = N + 1
MAGIC = 0x5f375a86
RSD = float(1.0 / np.sqrt(D))     # 1/sqrt(D)
RSD4 = float(D ** -0.25)          # D**-1/4 (Square scale for ua^2/sqrt(D))


def _rsqrt(nc, pool, x, P, W, pfx):
    """x**-0.5 on DVE: bit trick + 1 Newton iteration. rel err ~1.8e-3."""
    y0 = pool.tile([P, W], F32, tag=pfx + "_y0")
    nc.vector.tensor_scalar(y0.bitcast(I32)[:], x.bitcast(I32)[:], 1, None,
                            op0=ALU.logical_shift_right)
    nc.vector.tensor_scalar(y0.bitcast(I32)[:], y0.bitcast(I32)[:], -1, MAGIC,
                            op0=ALU.mult, op1=ALU.add)
    t = pool.tile([P, W], F32, tag=pfx + "_t")
    nc.vector.tensor_mul(t[:], y0[:], y0[:])
    u = pool.tile([P, W], F32, tag=pfx + "_u")
    nc.vector.scalar_tensor_tensor(u[:], t[:], 0.5, x[:], op0=ALU.mult, op1=ALU.mult)
    v = pool.tile([P, W], F32, tag=pfx + "_v")
    nc.vector.tensor_mul(v[:], u[:], y0[:])
    r = pool.tile([P, W], F32, tag=pfx + "_r")
    nc.vector.scalar_tensor_tensor(r[:], y0[:], 1.5, v[:], op0=ALU.mult, op1=ALU.subtract)
    return r


def _lrelu(nc, out_ap, in_ap):
    nc.vector.scalar_tensor_tensor(out_ap, in_ap, SLOPE, in_ap, op0=ALU.mult, op1=ALU.max)


def _pin_act_table(nc):
    """Restrict the ACT table-set chooser to natural_log_exp_and_others,
    which contains every activation this kernel uses (exp, ln, square,
    copy, prelu, identity) -> exactly one ACT_TABLE_LOAD. Other sets are
    emptied in place (indices must stay aligned with act_info.json)."""
    from concourse.bacc import get_activation_tables
    tabs = get_activation_tables(nc.m.arch)
    keep = "natural_log_exp_and_others"
    assert keep in tabs
    for name in tabs:
        if name != keep:
            tabs[name] = set()


def build():
    nc = bacc.Bacc()
    _pin_act_table(nc)
    emb = nc.dram_tensor("emb", [B_LOC, NODE, D], F32, kind="ExternalInput")
    # cstT cols: 0 = vq*sqrt(D) (rows 0:64), 1 = vk*sqrt(D) (rows 0:64),
    #            2 = [vi ; c0/64] (rows 0:128)
    cstT = nc.dram_tensor("cstT", [128, 4], F32, kind="ExternalInput")
    cstR = nc.dram_tensor("cstR", [1, 2 * D], F32, kind="ExternalInput")  # [g|b]
    out = nc.dram_tensor("out", [B_LOC, OUT_ROWS, D], F32, kind="ExternalOutput")

    with tile.TileContext(nc) as tc:
        with (
            tc.tile_pool(name="const", bufs=1) as cpool,
            tc.tile_pool(name="work", bufs=2) as pool,
            tc.tile_pool(name="psmall", bufs=2, space="PSUM") as psmall,
            tc.tile_pool(name="pqk", bufs=2, space="PSUM") as pqk,
            tc.tile_pool(name="pgram", bufs=1, space="PSUM") as pgram,
            tc.tile_pool(name="ps", bufs=2, space="PSUM") as ps,
        ):
            # ---- input DMAs: item tiles gate the LN pipeline -> first on
            # sync, then the consts. U rows on gpsimd AFTER make_identity
            # (DMA dispatch is ~0.6us of queue time; identity must not wait).
            in_tiles = []
            for e in range(B_LOC):
                tAB = pool.tile([128, 2, D], F32, tag=f"tAB{e}")
                nc.sync.dma_start(tAB[:], emb[e, 2:258, :].rearrange("(p n) d -> p n d", n=2))
                in_tiles.append(tAB)
            cst_sb = cpool.tile([1, 2 * D], F32)
            nc.sync.dma_start(cst_sb[:], cstR[:, :])
            cstT_f = cpool.tile([128, 4], F32)
            nc.sync.dma_start(cstT_f[:], cstT[:, :])

            # ---- global constants (identity first: gpsimd iota, then the
            # U-row DMAs ride behind it on the gpsimd queue) ----
            identF = cpool.tile([128, 128], F32)
            masks.make_identity(nc, identF[:])
            tU4 = cpool.tile([128, D], F32)
            u4v = tU4[:].rearrange("(a b) d -> a b d", b=32)
            nc.gpsimd.dma_start(u4v[0:2, 0:1, :], emb[0, 0:2, :])
            nc.gpsimd.dma_start(u4v[2:4, 0:1, :], emb[1, 0:2, :])
            identB = cpool.tile([128, 128], BF16)
            nc.scalar.copy(identB[:], identF[:])
            identR = cpool.tile([128, 128], F32R)
            nc.scalar.copy(identR[:], identF[:])
            ones_f = cpool.tile([1, 128], F32)
            nc.vector.memset(ones_f[:], 1.0)
            ones_r = cpool.tile([1, 128], F32R)
            nc.scalar.copy(ones_r[:], ones_f[:])
            ones_cb = cpool.tile([128, 2], BF16)
            nc.vector.memset(ones_cb[:], 1.0)
            ones64 = cpool.tile([64, 128], BF16)
            nc.vector.memset(ones64[:], 1.0)
            eps_col = cpool.tile([128, 1], F32)
            nc.vector.memset(eps_col[:], EPS)

            cstT_b = cpool.tile([128, 4], BF16)
            nc.scalar.copy(cstT_b[:], cstT_f[:])
            vq_bc = cpool.tile([64, 128], BF16)
            nc.vector.tensor_scalar_mul(vq_bc[:], ones64[:], cstT_f[0:64, 0:1])

            gb_rowr = cpool.tile([1, 2 * D], F32R)
            nc.scalar.copy(gb_rowr[:], cst_sb[:])
            p_gb = psmall.tile([128, 2 * D], F32, tag="small")
            nc.tensor.matmul(p_gb[:], ones_r[:], gb_rowr[:])
            gb_bc = cpool.tile([128, 2 * D], F32)
            nc.scalar.copy(gb_bc[:], p_gb[:])
            g_bc = gb_bc[:, 0:D]
            b_bc = gb_bc[:, D:2 * D]

            st = [dict() for _ in range(B_LOC)]

            # ---- LN stats via the DVE BN_STATS path: mean+var per row in two
            # DVE ops per tile, no ACT squares/accumulator reads at all.
            mv = pool.tile([128, 5, 2], F32, tag="mv")
            for e in range(B_LOC):
                for n in range(2):
                    bs = pool.tile([128, 6], F32, tag=f"bs{e}{n}")
                    nc.vector.bn_stats(bs[:], in_tiles[e][:, n, :])
                    nc.vector.bn_aggr(mv[:, 2 * e + n, :], bs[:])
            bsU = pool.tile([128, 6], F32, tag="bsU")
            nc.vector.bn_stats(bsU[:], tU4[:])
            nc.vector.bn_aggr(mv[:, 4, :], bsU[:])
            xv_all = pool.tile([128, 5], F32, tag="xv_all")
            nc.vector.tensor_scalar_add(xv_all[:], mv[:, :, 1], EPS)
            rstd_all = _rsqrt(nc, pool, xv_all, 128, 5, "lnr")

            xcU = pool.tile([128, D], F32, tag="xcU")
            nc.vector.tensor_scalar(xcU[:], tU4[:], mv[:, 4, 0:1], None, op0=ALU.subtract)
            xcs = []
            for e in range(B_LOC):
                for n in range(2):
                    xc = pool.tile([128, D], F32, tag=f"xc{e}{n}")
                    nc.vector.tensor_scalar(xc[:], in_tiles[e][:, n, :],
                                            mv[:, 2 * e + n, 0:1], None, op0=ALU.subtract)
                    xcs.append(xc)

            # ---- U rows LN -> elnU4 (quadrant rows: u0@0, i0@32, u1@64, i1@96)
            elnU4 = cpool.tile([128, D], F32R)
            nc.vector.scalar_tensor_tensor(elnU4[:], xcU[:], rstd_all[:, 4:5],
                                           g_bc, op0=ALU.mult, op1=ALU.mult)
            nc.vector.tensor_add(elnU4[:], elnU4.bitcast(F32)[:], b_bc)
            # columns of all four U rows at partitions 0:64 (for iid_bc)
            p_uT = psmall.tile([64, 128], F32R, tag="small")
            nc.tensor.transpose(p_uT[:], elnU4[:], identR[:])
            uT4 = cpool.tile([64, 128], F32)
            nc.scalar.copy(uT4[:], p_uT.bitcast(F32)[:])

            # ================= per-example pass A: ua + stack =================
            for e in range(B_LOC):
                S = st[e]
                if e == 0:
                    u0row = elnU4[0:1, :]
                else:
                    u0r = pool.tile([1, D], F32R, tag="u0r")
                    nc.scalar.copy(u0r[:], elnU4.bitcast(F32)[64:65, :])
                    u0row = u0r[:]
                S["u0row"] = u0row
                p_u0 = psmall.tile([128, D], F32, tag="small")
                nc.tensor.matmul(p_u0[:], ones_r[:], u0row)
                gu = pool.tile([128, D], BF16, tag="gu")
                nc.vector.tensor_mul(gu[:], g_bc, p_u0[:])
                bu = pool.tile([128, D], BF16, tag="bu")
                nc.vector.tensor_mul(bu[:], b_bc, p_u0[:])

                ua_both = pool.tile([128, 2, D], BF16, tag="ua_both")
                for n in range(2):
                    nc.vector.scalar_tensor_tensor(ua_both[:, n, :], xcs[2 * e + n][:],
                                                   rstd_all[:, 2 * e + n:2 * e + n + 1],
                                                   gu[:], op0=ALU.mult, op1=ALU.mult)
                    nc.vector.tensor_add(ua_both[:, n, :], ua_both[:, n, :], bu[:])
                S["ua_both"] = ua_both

                p_t = psmall.tile([64, N], BF16, tag="small")
                nc.tensor.transpose(p_t[:, 0:128], ua_both[:, 0, :], identB[:])
                nc.tensor.transpose(p_t[:, 128:256], ua_both[:, 1, :], identB[:])
                # stack rows 0:64 = uaT/sqrt(D); rows 64:128 = uaT^2/sqrt(D)
                stack = pool.tile([128, N], BF16, tag=f"stack{e}")
                nc.scalar.activation(stack[0:64, :], p_t[:], ACTF.Copy, scale=RSD)
                nc.scalar.activation(stack[64:128, :], p_t[:], ACTF.Square, scale=RSD4)
                S["stack"] = stack

                iid_bc = pool.tile([128, 128], BF16, tag=f"iidones{e}")
                nc.vector.tensor_scalar_mul(iid_bc[0:64, :], ones64[:],
                                            uT4[:, 32 + 64 * e:33 + 64 * e])
                nc.vector.memset(iid_bc[64:128, :], 1.0)
                S["iid_bc"] = iid_bc

            # ============ pass A2: scores + grams + rstd ====
            for e in range(B_LOC):
                S = st[e]
                stack = S["stack"]

                # s_k + s_i + c0, one PSUM column per j-block
                p_sk2 = psmall.tile([128, 2], F32, tag="small")
                for J in range(2):
                    cs = slice(J * 128, (J + 1) * 128)
                    nc.tensor.matmul(p_sk2[:, J:J + 1], stack[0:64, cs],
                                     cstT_b[0:64, 1:2], start=True, stop=False)
                    nc.tensor.matmul(p_sk2[:, J:J + 1], S["iid_bc"][:],
                                     cstT_b[:, 2:3], start=False, stop=True)
                sk_sb = pool.tile([128, 2], F32, tag="sk_sb")
                nc.scalar.copy(sk_sb[:], p_sk2[:])

                # s_q broadcast: [128, N] PSUM, row-invariant
                p_qkT = pqk.tile([128, N], F32, tag="qk")
                nc.tensor.matmul(p_qkT[:], vq_bc[:], stack[0:64, :])

                # qk^T blocks: Prelu per j-block (per-block bias), one fused
                # [128, 2N] Exp -> bf16 for both blocks
                qkT = pool.tile([128, 2 * N], F32, tag="qkT")
                for J in range(2):
                    nc.scalar.activation(qkT[:, J * N:(J + 1) * N], p_qkT[:], ACTF.Prelu,
                                         bias=sk_sb[:, J:J + 1], alpha=SLOPE)
                expvT = pool.tile([128, 2 * N], BF16, tag=f"expvT{e}")
                nc.scalar.activation(expvT[:], qkT[:], ACTF.Exp)
                S["expvT"] = expvT

                # softmax denominators as columns: denom[i] = sum_j expT[j, i]
                rden_cols = pool.tile([128, 2], F32, tag=f"rdenc{e}")
                for blk in range(2):
                    cs = slice(blk * 128, (blk + 1) * 128)
                    p_denc = psmall.tile([128, 2], F32, tag="small")
                    nc.tensor.matmul(p_denc[:], expvT[:, cs], ones_cb[:],
                                     start=True, stop=False)
                    nc.tensor.matmul(p_denc[:], expvT[:, N + cs.start:N + cs.stop],
                                     ones_cb[:], start=False, stop=True)
                    nc.vector.reciprocal(rden_cols[:, blk:blk + 1], p_denc[:, 0:1])
                S["rden_cols"] = rden_cols

                # gram matmuls: mu on PE rows 0:63, E2 on rows 64:127
                # (row-group concurrent); both blocks into one PSUM bank each
                var_big = pool.tile([128, 2 * N], F32, tag=f"var{e}")
                p_mu = pgram.tile([128, 2 * N], F32, tag="gmu")
                p_e2 = pgram.tile([128, 2 * N], F32, tag="ge2")
                for blk in range(2):
                    cs = slice(blk * 128, (blk + 1) * 128)
                    ns = slice(blk * N, (blk + 1) * N)
                    nc.tensor.matmul(p_mu[:, ns], stack[0:64, cs], stack[0:64, :])
                    nc.tensor.matmul(p_e2[:, ns], stack[64:128, cs], stack[64:128, :])
                msq = pool.tile([128, 2 * N], BF16, tag="msq")
                nc.scalar.activation(msq[:], p_mu[:], ACTF.Square)
                nc.vector.scalar_tensor_tensor(var_big[:], p_e2[:], 1.0,
                                               msq[:], op0=ALU.mult, op1=ALU.subtract)
                # rstd = exp(-0.5 * ln(var + EPS)); Ln+Exp share one table set
                lnv = pool.tile([128, 2 * N], F32, tag="lnv")
                nc.scalar.activation(lnv[:], var_big[:], ACTF.Ln, bias=eps_col[:])
                rstd = pool.tile([128, 2 * N], BF16, tag=f"rstd{e}")
                nc.scalar.activation(rstd[:], lnv[:], ACTF.Exp, scale=-0.5)
                S["rstd"] = rstd

            # ================= pass B: attention + output =================
            for e in range(B_LOC):
                S = st[e]
                ua_both = S["ua_both"]

                btT = pool.tile([128, 2 * N], BF16, tag="btT")
                nc.vector.tensor_mul(btT[:], S["expvT"][:], S["rstd"][:])

                p_S2 = ps.tile([128, 2, D], F32, tag="S")
                for blk in range(2):
                    cs = slice(blk * 128, (blk + 1) * 128)
                    nc.tensor.matmul(p_S2[:, blk, :], btT[:, cs], ua_both[:, 0, :],
                                     start=True, stop=False)
                    nc.tensor.matmul(p_S2[:, blk, :], btT[:, N + cs.start:N + cs.stop],
                                     ua_both[:, 1, :], start=False, stop=True)

                Sn = pool.tile([128, 2, D], BF16, tag="Sn")
                for blk in range(2):
                    nc.vector.tensor_scalar_mul(Sn[:, blk, :], p_S2[:, blk, :],
                                                S["rden_cols"][:, blk:blk + 1])
                t1n = pool.tile([128, 2, D], BF16, tag="t1n")
                nc.vector.tensor_mul(t1n[:], ua_both[:], Sn[:])
                c_raw = pool.tile([128, 2], F32, tag="c_raw")
                nc.vector.reduce_sum(c_raw[:], t1n[:], axis=AX.X)
                cn = pool.tile([128, 2], F32, tag="cn")
                nc.vector.tensor_scalar_mul(cn[:], c_raw[:], 1.0 / D)

                o_big = pool.tile([128, 2, D], F32, tag="o_big")
                for blk in range(2):
                    t2 = pool.tile([128, D], BF16, tag="t2")
                    nc.vector.scalar_tensor_tensor(t2[:], t1n[:, blk, :],
                                                   cn[:, blk:blk + 1], g_bc,
                                                   op0=ALU.subtract, op1=ALU.mult)
                    t3 = pool.tile([128, D], F32, tag="t3")
                    nc.vector.tensor_add(t3[:], t2[:], b_bc)
                    _lrelu(nc, o_big[:, blk, :], t3[:])
                out_rows = out[e, 1:257, :].rearrange("(p n) d -> p n d", n=2)
                (nc.sync if e == 0 else nc.gpsimd).dma_start(out_rows, o_big[:])

                iid_row = pool.tile([1, D], F32, tag="iid_row")
                nc.scalar.copy(iid_row[:], elnU4.bitcast(F32)[32 + 64 * e:33 + 64 * e, :])
                ui = pool.tile([1, D], F32, tag="ui")
                nc.vector.tensor_mul(ui[:], S["u0row"].bitcast(F32), iid_row[:])
                uo = pool.tile([1, D], F32, tag="uo")
                _lrelu(nc, uo[:], ui[:])
                (nc.sync if e == 0 else nc.gpsimd).dma_start(out[e, 0:1, :], uo[:])

    nc.compile()
    return nc


def _host_consts(Wa, ba, a_w, a_b):
    aq, ak, ai = a_w[:D], a_w[D:2 * D], a_w[2 * D:]
    sD = float(np.sqrt(D))
    vq = aq @ Wa * sD
    vk = ak @ Wa * sD
    vi = ai @ Wa
    c0 = float(ba @ aq + ba @ ak + ba @ ai + a_b[0])
    cstT = np.zeros((128, 4), np.float32)
    cstT[0:D, 0] = vq
    cstT[0:D, 1] = vk
    cstT[0:D, 2] = vi
    cstT[D:128, 2] = c0 / 64.0
    return cstT


_NC_CACHE = {}


def _get_nc():
    if "nc" not in _NC_CACHE:
        _NC_CACHE["nc"] = build()
    return _NC_CACHE["nc"]


def run(embeddings, Wa, ba, a_w, a_b, ln_g, ln_b, **spmd_kwargs):
    embeddings = np.ascontiguousarray(embeddings, dtype=np.float32)
    cstT = _host_consts(np.asarray(Wa, np.float32), np.asarray(ba, np.float32),
                        np.asarray(a_w, np.float32), np.asarray(a_b, np.float32))
    cstR = np.zeros((1, 2 * D), np.float32)
    cstR[0, 0:D] = np.asarray(ln_g, np.float32)
    cstR[0, D:2 * D] = np.asarray(ln_b, np.float32)

    nc = _get_nc()
    in_maps = [
        {"emb": embeddings[c * B_LOC:(c + 1) * B_LOC], "cstT": cstT, "cstR": cstR}
        for c in range(N_CORES)
    ]
    res = run_bass_kernel_spmd(nc, in_maps, core_ids=list(range(N_CORES)), **spmd_kwargs)
    outp = np.concatenate([res.results[c]["out"] for c in range(N_CORES)], axis=0)
    return outp, res


def kernel(embeddings, Wa, ba, a_w, a_b, ln_g, ln_b):
    outp, _ = run(embeddings, Wa, ba, a_w, a_b, ln_g, ln_b)
    return outp
